# revision 7
# baseline (speedup 1.0000x reference)
"""DenseGCN Trainium2 kernel (8-core SPMD), v2.

Strategy (1D node partition by dst; GPSIMD ap_gather message pipeline):
- Nodes range-sharded: core c owns dsts [c*8192, (c+1)*8192).
- h replicated per layer via AllGather of FEATURE-MAJOR bf16 shards
  (hT [64, 8192]); each core loads the full table into SBUF as
  [128 part, 32768] bf16 = packed node-PAIR f32 words: partitions 0-63
  hold features of nodes 0..32767 (lo), 64-127 the hi half.
- Edges assigned to the dst core, split lo/hi by src half, sorted by
  dst, per-dst padded to a multiple of 4 slots, grouped by 512-dst
  groups, each (group, half) padded to K*512 slots (K = global max).
- Per group: one ap_gather (Q7 compute gather, idx = src>>1 packed
  pairs) pulls h pair-words for lo (parts 0-63) and hi (parts 64-127)
  streams at once; DVE multiplies the bitcast bf16 stream by a static
  ewp mask (ew * parity one-hot) in place, then one segmented
  tensor_reduce (window 8 = 4 edges x 2 pair slots) produces merged
  messages msgM [128, K*128] f32 (lo rows 0-63 / hi 64-127).
- Per 128-merged-slot tile: PE transpose -> msgT bf16; two one-hot
  scatter matmuls (static cm [128, 512] 0/1 bf16 from DRAM) accumulate
  aggT [64 feat, 512 dst] in PSUM (exact f32).
- Conv + LN + residual per 128-node tile from aggT; epilogue also
  maintains the next layer's hT shard via PE transposes.
"""

import math

import numpy as np
import ml_dtypes

import concourse.bacc as bacc
import concourse.mybir as mybir
import concourse.tile as tile
from concourse import library_config
from concourse.bass_utils import run_bass_kernel_spmd

F32 = mybir.dt.float32
BF16 = mybir.dt.bfloat16
I16 = mybir.dt.int16
AF = mybir.ActivationFunctionType
ALU = mybir.AluOpType
AX = mybir.AxisListType

NC_CORES = 8
F_IN = 128
H = 64
OUT = 32
L = 3
EPS = 1e-5
P = 128

NG = 16          # dst groups per core
GD = 512         # dsts per group
PAD = 4          # per-dst edge-slot quantum
WIN = 2 * PAD    # segreduce window (pair slots)
NPBF = np.dtype(ml_dtypes.bfloat16)


def _wrap16(idx, reps):
    """[n] -> [16*reps, n/16] int16; idx j lands at [j%16, j//16]."""
    n = len(idx)
    assert n % 16 == 0
    w = idx.reshape(n // 16, 16).T.astype(np.int16)
    return np.ascontiguousarray(np.tile(w, (reps, 1)))


def prep_inputs(x, edge_weight, src, dst, n_nodes, npc):
    """Host-side shard + slot construction. Returns per-core maps, K."""
    ew = edge_weight.reshape(-1).astype(np.float32)
    src = src.astype(np.int64)
    dst = dst.astype(np.int64)
    half = n_nodes // 2

    # pass 1: per (core, half) sorted edge arrays + K
    packs = []
    kmax = 1
    for c in range(NC_CORES):
        m = (dst // npc) == c
        s_c, d_c, w_c = src[m], dst[m] - c * npc, ew[m]
        halves = []
        for h in range(2):
            hm = (s_c >= half) == bool(h)
            s = s_c[hm] - h * half
            dl = d_c[hm]
            w = w_c[hm]
            order = np.argsort(dl, kind="stable")
            s, dl, w = s[order], dl[order], w[order]
            cnt = np.bincount(dl, minlength=npc)
            mcnt = (cnt + PAD - 1) // PAD  # merged slots per dst
            gm = mcnt.reshape(NG, GD).sum(1)
            kmax = max(kmax, math.ceil(int(gm.max()) / P))
            halves.append((s, dl, w, cnt, mcnt))
        packs.append(halves)
    K = kmax
    S = NG * K * GD          # gather slots per (core, half)
    MG = K * P               # merged slots per (group, half)

    maps = []
    for c in range(NC_CORES):
        out = {
            "x": np.ascontiguousarray(x[c * npc : (c + 1) * npc]).astype(
                np.float32
            )
        }
        gidx128 = np.zeros((P, S // 16), np.int16)
        ewp = np.zeros((2, 2 * S), np.float32)
        cms = np.zeros((NG * K, 2, P, GD), np.float32)
        for h in range(2):
            s, dl, w, cnt, mcnt = packs[c][h]
            # merged-slot start per dst (group-padded layout)
            pre = np.cumsum(mcnt) - mcnt          # global prefix
            g_of_d = np.arange(npc) // GD
            gbase_pre = pre[g_of_d * GD]          # prefix at group start
            mstart = g_of_d * MG + (pre - gbase_pre)
            # edge gather-slot positions
            estart = np.cumsum(cnt) - cnt
            rank = np.arange(len(dl)) - estart[dl]
            slot = mstart[dl] * PAD + rank
            gi = np.zeros(S, np.int64)
            gi[slot] = s >> 1
            ewp[h, 2 * slot + (s & 1)] = w
            # merged dst map
            tot = int(mcnt.sum())
            mpre = np.cumsum(mcnt) - mcnt
            within = np.arange(tot) - np.repeat(mpre, mcnt)
            pos = np.repeat(mstart, mcnt) + within
            msd = np.full(NG * MG, -1, np.int64)
            msd[pos] = np.repeat(np.arange(npc) % GD, mcnt)
            # one-hot cms
            msd_r = msd.reshape(NG * K, P)
            onehot = msd_r[:, :, None] == np.arange(GD)[None, None, :]
            cms[:, h] = onehot.astype(np.float32)
            # idx wrap: parts [64h, 64h+64)
            w16 = _wrap16(gi, 4)
            gidx128[64 * h : 64 * (h + 1)] = w16
        out["gidx"] = gidx128
        out["ewp"] = ewp.astype(NPBF)
        out["cms"] = cms.reshape(NG * K * 2, P, GD).astype(NPBF)
        maps.append(out)
    return maps, K


def build_nc(n_nodes, npc, K, ln_identity):
    nc = bacc.Bacc(None, target_bir_lowering=False)
    half = n_nodes // 2
    ntile = npc // P         # node tiles per core (64)
    S = NG * K * GD
    SLOT_G = K * GD          # gather idxs per group
    MG = K * P               # merged slots per group

    # ---- I/O ----
    x_d = nc.declare_dram_parameter("x", [npc, F_IN], F32, isOutput=False)
    gidx_d = nc.declare_dram_parameter("gidx", [P, S // 16], I16, isOutput=False)
    ewp_d = nc.declare_dram_parameter("ewp", [2, 2 * S], BF16, isOutput=False)
    cms_d = nc.declare_dram_parameter(
        "cms", [NG * K * 2, P, GD], BF16, isOutput=False
    )
    w1_d = nc.declare_dram_parameter("w1", [F_IN, H], F32, isOutput=False)
    b1_d = nc.declare_dram_parameter("b1r", [P, H], F32, isOutput=False)
    cw_d = [
        nc.declare_dram_parameter(f"cw{i}", [H, H], BF16, isOutput=False)
        for i in range(L)
    ]
    cb_d = [
        nc.declare_dram_parameter(f"cb{i}r", [P, H], F32, isOutput=False)
        for i in range(L)
    ]
    w3_d = nc.declare_dram_parameter("w3", [H, H], F32, isOutput=False)
    b3_d = nc.declare_dram_parameter("b3r", [P, H], F32, isOutput=False)
    w4_d = nc.declare_dram_parameter("w4", [H, OUT], F32, isOutput=False)
    b4_d = nc.declare_dram_parameter("b4r", [P, OUT], F32, isOutput=False)
    ident_d = nc.declare_dram_parameter("ident", [P, P], F32, isOutput=False)
    ln_d = {}
    if not ln_identity:
        ln_d["ln1g"] = nc.declare_dram_parameter("ln1g", [P, F_IN], F32, False)
        ln_d["ln1b"] = nc.declare_dram_parameter("ln1b", [P, F_IN], F32, False)
        ln_d["lng"] = nc.declare_dram_parameter("lng", [P, H], F32, False)
        ln_d["lnb"] = nc.declare_dram_parameter("lnb", [P, H], F32, False)
        ln_d["ln2g"] = nc.declare_dram_parameter("ln2g", [P, H], F32, False)
        ln_d["ln2b"] = nc.declare_dram_parameter("ln2b", [P, H], F32, False)
    out_d = nc.declare_dram_parameter("out", [npc, OUT], F32, isOutput=True)

    # ---- internal DRAM ----
    hT_bounce = nc.dram_tensor("hT_bounce", [H, npc], BF16)
    hfull_d = nc.dram_tensor(
        "hfull", [NC_CORES * H, npc], BF16, addr_space="Shared"
    )
    groups_all = [list(range(NC_CORES))]

    with tile.TileContext(nc) as tc:
        with (
            tc.tile_pool(name="const", bufs=1) as cpool,
            tc.tile_pool(name="gx", bufs=2) as gxpool,
            tc.tile_pool(name="ewp", bufs=1) as ewpool,
            tc.tile_pool(name="gi", bufs=2) as gipool,
            tc.tile_pool(name="msg", bufs=2) as mpool,
            tc.tile_pool(name="cm", bufs=3) as cmpool,
            tc.tile_pool(name="mt", bufs=3) as mtpool,
            tc.tile_pool(name="work", bufs=2) as wpool,
            tc.tile_pool(name="stat", bufs=8) as spool,
            tc.tile_pool(name="psA", bufs=2, space="PSUM") as psA,
            tc.tile_pool(name="psB", bufs=2, space="PSUM") as psB,
            tc.tile_pool(name="psC", bufs=2, space="PSUM") as psC,
        ):
            nc.gpsimd.load_library(library_config.ap_gather)

            ident = cpool.tile([P, P], F32)
            nc.sync.dma_start(out=ident[:], in_=ident_d[:, :])
            w1_s = cpool.tile([F_IN, H], F32)
            nc.sync.dma_start(out=w1_s[:], in_=w1_d[:, :])
            b1_s = cpool.tile([P, H], F32)
            nc.sync.dma_start(out=b1_s[:], in_=b1_d[:, :])
            cw_s, cb_s = [], []
            for i in range(L):
                w = cpool.tile([H, H], BF16, tag=f"cw{i}")
                nc.sync.dma_start(out=w[:], in_=cw_d[i][:, :])
                cw_s.append(w)
                b = cpool.tile([P, H], F32, tag=f"cb{i}")
                nc.sync.dma_start(out=b[:], in_=cb_d[i][:, :])
                cb_s.append(b)
            w3_s = cpool.tile([H, H], F32, tag="w3")
            nc.sync.dma_start(out=w3_s[:], in_=w3_d[:, :])
            b3_s = cpool.tile([P, H], F32, tag="b3")
            nc.sync.dma_start(out=b3_s[:], in_=b3_d[:, :])
            w4_s = cpool.tile([H, OUT], F32, tag="w4")
            nc.sync.dma_start(out=w4_s[:], in_=w4_d[:, :])
            b4_s = cpool.tile([P, OUT], F32, tag="b4")
            nc.sync.dma_start(out=b4_s[:], in_=b4_d[:, :])
            ln_s = {}
            for k in ln_d:
                f = F_IN if k.startswith("ln1") else H
                t_ = cpool.tile([P, f], F32, tag=k)
                nc.sync.dma_start(out=t_[:], in_=ln_d[k][:, :])
                ln_s[k] = t_

            htab = cpool.tile([P, half], BF16, tag="htab")  # 64KB/part
            hT_shard = cpool.tile([H, npc], BF16, tag="htsh")
            h_stage = cpool.tile([P, ntile * H], F32, tag="hstage")
            out_stage = cpool.tile([P, ntile * OUT], F32, tag="ostage")

            # ---------- helpers (same math as v1) ----------
            def layer_norm(dst_ap, src_ap, f, gkey=None, bkey=None):
                parts = src_ap.shape[0]
                ssum = spool.tile([P, 1], F32, tag="lnsum")
                nc.vector.tensor_reduce(
                    out=ssum[:parts], in_=src_ap, axis=AX.X, op=ALU.add
                )
                xc = wpool.tile([P, f], F32, tag=f"lnxc{f}")
                nc.vector.tensor_scalar(
                    out=xc[:parts],
                    in0=src_ap,
                    scalar1=float(f),
                    scalar2=ssum[:parts, 0:1],
                    op0=ALU.mult,
                    op1=ALU.subtract,
                )
                sq = wpool.tile([P, f], F32, tag=f"lnsq{f}")
                nc.vector.tensor_tensor(
                    out=sq[:parts], in0=xc[:parts], in1=xc[:parts], op=ALU.mult
                )
                vsum = spool.tile([P, 1], F32, tag="lnvar")
                nc.vector.tensor_reduce(
                    out=vsum[:parts], in_=sq[:parts], axis=AX.X, op=ALU.add
                )
                veps = spool.tile([P, 1], F32, tag="lnveps")
                nc.vector.tensor_scalar(
                    out=veps[:parts],
                    in0=vsum[:parts],
                    scalar1=1.0 / f,
                    scalar2=float(f) * float(f) * EPS,
                    op0=ALU.mult,
                    op1=ALU.add,
                )
                std = spool.tile([P, 1], F32, tag="lnstd")
                nc.scalar.sqrt(std[:parts], veps[:parts])
                rstd = spool.tile([P, 1], F32, tag="lnrstd")
                nc.vector.reciprocal(rstd[:parts], std[:parts])
                if gkey is None:
                    nc.vector.tensor_scalar(
                        out=dst_ap,
                        in0=xc[:parts],
                        scalar1=rstd[:parts, 0:1],
                        scalar2=None,
                        op0=ALU.mult,
                    )
                else:
                    nrm = wpool.tile([P, f], F32, tag=f"lnnrm{f}")
                    nc.vector.tensor_scalar(
                        out=nrm[:parts],
                        in0=xc[:parts],
                        scalar1=rstd[:parts, 0:1],
                        scalar2=None,
                        op0=ALU.mult,
                    )
                    tmp = wpool.tile([P, f], F32, tag=f"lnaf{f}")
                    nc.vector.tensor_tensor(
                        out=tmp[:parts],
                        in0=nrm[:parts],
                        in1=ln_s[gkey][:parts],
                        op=ALU.mult,
                    )
                    nc.vector.tensor_tensor(
                        out=dst_ap,
                        in0=tmp[:parts],
                        in1=ln_s[bkey][:parts],
                        op=ALU.add,
                    )

            def elu(dst_ap, src_ap, f):
                parts = src_ap.shape[0]
                r1 = wpool.tile([P, f], F32, tag=f"elur{f}")
                nc.vector.tensor_scalar(
                    out=r1[:parts],
                    in0=src_ap,
                    scalar1=0.0,
                    scalar2=1.0,
                    op0=ALU.max,
                    op1=ALU.subtract,
                )
                mn = wpool.tile([P, f], F32, tag=f"elum{f}")
                nc.vector.tensor_scalar(
                    out=mn[:parts],
                    in0=src_ap,
                    scalar1=0.0,
                    scalar2=None,
                    op0=ALU.min,
                )
                ex = wpool.tile([P, f], F32, tag=f"elue{f}")
                nc.scalar.activation(ex[:parts], mn[:parts], AF.Exp)
                nc.vector.tensor_tensor(
                    out=dst_ap, in0=r1[:parts], in1=ex[:parts], op=ALU.add
                )

            def stage_to_hT(t):
                """hT_shard[:, t*128:+128] = bf16(h_stage tile t)^T."""
                sl = slice(t * H, (t + 1) * H)
                trp = psB.tile([H, P], F32, tag="trp")
                nc.tensor.transpose(
                    out=trp[:], in_=h_stage[:, sl], identity=ident[:]
                )
                nc.vector.tensor_copy(
                    hT_shard[:, t * P : (t + 1) * P], trp[:]
                )

            # ---------- fc_first ----------
            for t in range(ntile):
                xt = wpool.tile([P, F_IN], F32, tag="xt")
                nc.sync.dma_start(out=xt[:], in_=x_d[t * P : (t + 1) * P, :])
                lnx = wpool.tile([P, F_IN], F32, tag="lnx")
                if ln_identity:
                    layer_norm(lnx[:], xt[:], F_IN)
                else:
                    layer_norm(lnx[:], xt[:], F_IN, "ln1g", "ln1b")
                xT_ps = psB.tile([P, P], F32, tag="trp")
                nc.tensor.transpose(out=xT_ps[:], in_=lnx[:], identity=ident[:])
                xT = wpool.tile([P, P], F32, tag="xT")
                nc.vector.tensor_copy(xT[:], xT_ps[:])
                h_ps = psC.tile([P, H], F32, tag="linps")
                nc.tensor.matmul(
                    out=h_ps[:], lhsT=xT[:], rhs=w1_s[:], start=True, stop=True
                )
                hb = wpool.tile([P, H], F32, tag="hb")
                nc.vector.tensor_tensor(
                    out=hb[:], in0=h_ps[:], in1=b1_s[:], op=ALU.add
                )
                he = wpool.tile([P, H], F32, tag="he")
                elu(he[:], hb[:], H)
                sl = slice(t * H, (t + 1) * H)
                if ln_identity:
                    layer_norm(h_stage[:, sl], he[:], H)
                else:
                    layer_norm(h_stage[:, sl], he[:], H, "lng", "lnb")
                stage_to_hT(t)

            # ---------- conv layers ----------
            for li in range(L):
                nc.sync.dma_start(out=hT_bounce[:, :], in_=hT_shard[:])
                nc.gpsimd.collective_compute(
                    "AllGather",
                    ALU.bypass,
                    replica_groups=groups_all,
                    ins=[hT_bounce[:, :]],
                    outs=[hfull_d[:, :]],
                )
                # table load: parts 0-63 <- cores 0-3 (lo nodes), 64-127 hi
                hfv = hfull_d[:, :].rearrange("(k c) n -> k c n", c=H)
                nc.sync.dma_start(
                    out=htab[0:H, :].rearrange("c (k n) -> c k n", k=4),
                    in_=hfv[0:4].rearrange("k c n -> c k n"),
                )
                nc.sync.dma_start(
                    out=htab[H:P, :].rearrange("c (k n) -> c k n", k=4),
                    in_=hfv[4:8].rearrange("k c n -> c k n"),
                )
                tabv = htab[:].bitcast(F32).rearrange(
                    "p (n one) -> p n one", one=1
                )

                for g in range(NG):
                    gi_sb = gipool.tile([P, SLOT_G // 16], I16, tag="gi")
                    nc.sync.dma_start(
                        out=gi_sb[:],
                        in_=gidx_d[
                            :, g * (SLOT_G // 16) : (g + 1) * (SLOT_G // 16)
                        ],
                    )
                    ew_sb = ewpool.tile([P, 2 * SLOT_G], BF16, tag="ewp")
                    for hh in range(2):
                        nc.sync.dma_start(
                            out=ew_sb[64 * hh : 64 * hh + 64, :],
                            in_=ewp_d[
                                hh : hh + 1,
                                g * 2 * SLOT_G : (g + 1) * 2 * SLOT_G,
                            ].partition_broadcast(64),
                        )
                    gx = gxpool.tile([P, SLOT_G], F32, tag="gx")
                    nc.gpsimd.ap_gather(
                        out_ap=gx[:].rearrange("p (n one) -> p n one", one=1),
                        in_ap=tabv,
                        idxs_ap=gi_sb[:],
                        channels=P,
                        num_elems=half // 2,
                        d=1,
                        num_idxs=SLOT_G,
                    )
                    gxb = gx[:].bitcast(BF16)  # [P, 2*SLOT_G]
                    nc.vector.tensor_tensor(
                        out=gxb, in0=gxb, in1=ew_sb[:], op=ALU.mult
                    )
                    msgM = mpool.tile([P, MG], F32, tag="msgM")
                    nc.vector.tensor_reduce(
                        out=msgM[:],
                        in_=gxb.rearrange("p (m w) -> p m w", w=WIN),
                        axis=AX.X,
                        op=ALU.add,
                    )
                    aggps = psA.tile([H, GD], F32, tag="aggps")
                    for t in range(K):
                        cmt = cmpool.tile([P, 2 * GD], BF16, tag="cm")
                        nc.sync.dma_start(
                            out=cmt[:].rearrange("p (h j) -> p h j", h=2),
                            in_=cms_d[
                                (g * K + t) * 2 : (g * K + t) * 2 + 2
                            ].rearrange("h p j -> p h j"),
                        )
                        trp = psB.tile([P, P], F32, tag="trp")
                        nc.tensor.transpose(
                            out=trp[:],
                            in_=msgM[:, t * P : (t + 1) * P],
                            identity=ident[:],
                        )
                        msgT = mtpool.tile([P, P], BF16, tag="msgT")
                        nc.vector.tensor_copy(msgT[:], trp[:])
                        nc.tensor.matmul(
                            out=aggps[:],
                            lhsT=msgT[:, 0:H],
                            rhs=cmt[:, 0:GD],
                            start=(t == 0),
                            stop=False,
                        )
                        nc.tensor.matmul(
                            out=aggps[:],
                            lhsT=msgT[:, H:P],
                            rhs=cmt[:, GD : 2 * GD],
                            start=False,
                            stop=(t == K - 1),
                        )
                    aggsb = wpool.tile([H, GD], BF16, tag="aggsb")
                    nc.vector.tensor_copy(aggsb[:], aggps[:])
                    for q in range(GD // P):
                        nt = g * (GD // P) + q
                        lin = psC.tile([P, H], F32, tag="linps")
                        nc.tensor.matmul(
                            out=lin[:],
                            lhsT=aggsb[:, q * P : (q + 1) * P],
                            rhs=cw_s[li][:],
                            start=True,
                            stop=True,
                        )
                        zb = wpool.tile([P, H], F32, tag="hb")
                        nc.vector.tensor_tensor(
                            out=zb[:], in0=lin[:], in1=cb_s[li][:], op=ALU.add
                        )
                        hn = wpool.tile([P, H], F32, tag="hn")
                        if ln_identity:
                            layer_norm(hn[:], zb[:], H)
                        else:
                            layer_norm(hn[:], zb[:], H, "lng", "lnb")
                        gsl = slice(nt * H, (nt + 1) * H)
                        # res_sum == h at layer boundary: h_new = hn + h_old
                        nc.vector.tensor_tensor(
                            out=h_stage[:, gsl],
                            in0=hn[:],
                            in1=h_stage[:, gsl],
                            op=ALU.add,
                        )
                        if li < L - 1:
                            stage_to_hT(nt)

            # ---------- fc_final ----------
            for t in range(ntile):
                sl = slice(t * H, (t + 1) * H)
                lnh = wpool.tile([P, H], F32, tag="lnh")
                if ln_identity:
                    layer_norm(lnh[:], h_stage[:, sl], H)
                else:
                    layer_norm(lnh[:], h_stage[:, sl], H, "ln2g", "ln2b")
                tr_ps = psB.tile([H, P], F32, tag="trp")
                nc.tensor.transpose(out=tr_ps[:], in_=lnh[:], identity=ident[:])
                lnhT = wpool.tile([H, P], F32, tag="aggT")
                nc.vector.tensor_copy(lnhT[:], tr_ps[:])
                z_ps = psC.tile([P, H], F32, tag="linps")
                nc.tensor.matmul(
                    out=z_ps[:], lhsT=lnhT[:], rhs=w3_s[:], start=True, stop=True
                )
                zb = wpool.tile([P, H], F32, tag="hb")
                nc.vector.tensor_tensor(
                    out=zb[:], in0=z_ps[:], in1=b3_s[:], op=ALU.add
                )
                ze = wpool.tile([P, H], F32, tag="he")
                elu(ze[:], zb[:], H)
                tr2_ps = psB.tile([H, P], F32, tag="trp")
                nc.tensor.transpose(out=tr2_ps[:], in_=ze[:], identity=ident[:])
                zT = wpool.tile([H, P], F32, tag="aggT")
                nc.vector.tensor_copy(zT[:], tr2_ps[:])
                o_ps = psC.tile([P, OUT], F32, tag="ops")
                nc.tensor.matmul(
                    out=o_ps[:], lhsT=zT[:], rhs=w4_s[:], start=True, stop=True
                )
                osl = slice(t * OUT, (t + 1) * OUT)
                nc.vector.tensor_tensor(
                    out=out_stage[:, osl], in0=o_ps[:], in1=b4_s[:], op=ALU.add
                )

            out_v = out_d[:, :].rearrange("(t p) f -> p t f", p=P)
            nc.sync.dma_start(
                out=out_v,
                in_=out_stage[:].rearrange("p (t f) -> p t f", f=OUT),
            )

    nc.compile()
    return nc


def _replicate(v, parts=P):
    return np.ascontiguousarray(
        np.tile(np.asarray(v, np.float32)[None, :], (parts, 1))
    )


def kernel(
    x,
    edge_weight,
    src,
    dst,
    ln1_g,
    ln1_b,
    w1,
    b1,
    ln_g,
    ln_b,
    conv_w,
    conv_b,
    ln2_g,
    ln2_b,
    w3,
    b3,
    w4,
    b4,
    _n_cores=NC_CORES,
    _trace=False,
    _run_kwargs=None,
):
    x = np.asarray(x, np.float32)
    n_nodes = x.shape[0]
    npc = n_nodes // NC_CORES

    ln_identity = (
        np.all(ln1_g == 1) and np.all(ln1_b == 0)
        and np.all(ln_g == 1) and np.all(ln_b == 0)
        and np.all(ln2_g == 1) and np.all(ln2_b == 0)
    )

    maps, K = prep_inputs(
        x, np.asarray(edge_weight), np.asarray(src), np.asarray(dst),
        n_nodes, npc,
    )

    weights = {
        "ident": np.eye(P, dtype=np.float32),
        "w1": np.asarray(w1, np.float32),
        "b1r": _replicate(b1),
        "w3": np.asarray(w3, np.float32),
        "b3r": _replicate(b3),
        "w4": np.asarray(w4, np.float32),
        "b4r": _replicate(b4),
    }
    for i in range(L):
        weights[f"cw{i}"] = np.asarray(conv_w[i], np.float32).astype(NPBF)
        weights[f"cb{i}r"] = _replicate(conv_b[i])
    if not ln_identity:
        weights["ln1g"] = _replicate(ln1_g)
        weights["ln1b"] = _replicate(ln1_b)
        weights["lng"] = _replicate(ln_g)
        weights["lnb"] = _replicate(ln_b)
        weights["ln2g"] = _replicate(ln2_g)
        weights["ln2b"] = _replicate(ln2_b)

    in_maps = [{**m, **weights} for m in maps]

    nc = build_nc(n_nodes, npc, K, ln_identity)
    res = run_bass_kernel_spmd(
        nc, in_maps, core_ids=list(range(NC_CORES)), trace=_trace,
        **(_run_kwargs or {}),
    )
    global LAST_RESULTS
    LAST_RESULTS = res
    return np.concatenate([r["out"] for r in res.results], axis=0)


LAST_RESULTS = None
